# revision 10
# baseline (speedup 1.0000x reference)
"""DGCNN forward kernel for 8 Trainium2 NeuronCores (data-parallel over batch).

Five SPMD launches: one per EdgeConv layer (pairwise-dist matmul on PE,
exact top-10 via DVE max8/max_index/match_replace, neighbor gather via
gpsimd ap_gather, local BN partial sums) and one head launch
(cat -> conv5 -> conv6 -> dense1 -> dense2). Training-mode BN couples all
8 point clouds, so the tiny per-layer [O,2] partial sums are combined on
the host (float64) between launches.

EdgeConv algebra:
  h[n,kk,:] = Wnb x[idx[n,kk]] + (Wctr-Wnb) x[n]   (y-gather + z)
  max_k h = max_k(y[idx]) + z;  BN+LeakyReLU commute with max_k
  (BN scale gamma=1 > 0).
"""
import sys
import numpy as np

sys.path.insert(0, "/opt/trn_rl_repo")

from contextlib import ExitStack

import concourse.bass as bass
import concourse.bacc as bacc
import concourse.tile as tile
from concourse import mybir

F32 = mybir.dt.float32
I16 = mybir.dt.int16
U32 = mybir.dt.uint32

B = 8
N = 2048
NT = N // 128
KNN = 10
NQ = 4                   # gather quarters
TPQ = NT // NQ
NPQ = N // NQ
SQ = KNN * NPQ // 16     # wrapped idx cols per quarter
EPS = 1e-5
NEG = -1e30

LAYERS = [(24, 64), (64, 64), (64, 128), (128, 256)]


def _leaky(nc, out_ap, in_ap):
    nc.vector.scalar_tensor_tensor(out_ap, in_ap, 0.2, in_ap,
                                   mybir.AluOpType.mult, mybir.AluOpType.max)


def build_stage(L):
    C, O = LAYERS[L]
    OC = (O + 127) // 128
    OCW = [min(128, O - i * 128) for i in range(OC)]
    nc = bacc.Bacc("TRN2", target_bir_lowering=False, debug=False, num_devices=8)

    if L == 0:
        xin_d = nc.dram_tensor("xin", [C, N], F32, kind="ExternalInput")
    else:
        mh_d = nc.dram_tensor("maxh_prev", [C, N], F32, kind="ExternalInput")
        a_d = nc.dram_tensor("a_prev", [C, 1], F32, kind="ExternalInput")
        b_d = nc.dram_tensor("b_prev", [C, 1], F32, kind="ExternalInput")
    anb_d = nc.dram_tensor("anb", [C, O], F32, kind="ExternalInput")
    bz_d = nc.dram_tensor("bz", [C, O], F32, kind="ExternalInput")
    ident_d = nc.dram_tensor("ident", [128, 128], F32, kind="ExternalInput")

    maxh_out = nc.dram_tensor("maxh", [O, N], F32, kind="ExternalOutput")
    part_out = nc.dram_tensor("part", [O, 2], F32, kind="ExternalOutput")

    with tile.TileContext(nc) as tc, ExitStack() as ctx:
        per = ctx.enter_context(tc.tile_pool(name="per", bufs=1))
        wk = ctx.enter_context(tc.tile_pool(name="wk", bufs=1))
        dram = ctx.enter_context(tc.tile_pool(name="dram", bufs=1, space="DRAM"))
        ps = ctx.enter_context(tc.tile_pool(name="ps", bufs=2, space="PSUM"))

        x = per.tile([C, N], F32)
        if L == 0:
            nc.sync.dma_start(x[:], xin_d[:, :])
        else:
            mh = per.tile([C, N], F32)
            nc.sync.dma_start(mh[:], mh_d[:, :])
            av = per.tile([C, 1], F32)
            nc.sync.dma_start(av[:], a_d[:, :])
            bv = per.tile([C, 1], F32)
            nc.sync.dma_start(bv[:], b_d[:, :])
            xt = per.tile([C, N], F32)
            nc.vector.tensor_scalar(xt[:], mh[:], av[:], bv[:],
                                    mybir.AluOpType.mult, mybir.AluOpType.add)
            _leaky(nc, x[:], xt[:])

        anb = per.tile([C, O], F32)
        nc.sync.dma_start(anb[:], anb_d[:, :])
        bz = per.tile([C, O], F32)
        nc.sync.dma_start(bz[:], bz_d[:, :])
        ident = per.tile([128, 128], F32)
        nc.sync.dma_start(ident[:], ident_d[:, :])

        x2 = per.tile([C, N], F32)
        nc.vector.tensor_mul(x2[:], x[:], x[:])
        ones_c = per.tile([C, 1], F32)
        nc.vector.memset(ones_c[:], 1.0)
        ones_r = per.tile([1, 128], F32)
        nc.vector.memset(ones_r[:], 1.0)
        negxxh = per.tile([1, N], F32)
        for j in range(N // 512):
            pxx = ps.tile([1, 512], F32, tag="py", bufs=2)
            nc.tensor.matmul(pxx[:], ones_c[:],
                             x2[:, j*512:(j+1)*512], start=True, stop=True)
            nc.scalar.activation(negxxh[:, j*512:(j+1)*512], pxx[:],
                                 mybir.ActivationFunctionType.Copy, scale=-0.5)

        # y / z feature-major [O, N]
        yts, zts = [], []
        for oc in range(OC):
            ow = OCW[oc]
            yt = per.tile([128, N], F32, name=f"yt{oc}")
            zt = per.tile([ow, N], F32, name=f"zt{oc}")
            for j in range(N // 512):
                py = ps.tile([ow, 512], F32, tag="py", bufs=2)
                nc.tensor.matmul(py[:], anb[:, oc*128:oc*128+ow],
                                 x[:, j*512:(j+1)*512], start=True, stop=True)
                nc.scalar.copy(yt[:ow, j*512:(j+1)*512], py[:])
                pz = ps.tile([ow, 512], F32, tag="py", bufs=2)
                nc.tensor.matmul(pz[:], bz[:, oc*128:oc*128+ow],
                                 x[:, j*512:(j+1)*512], start=True, stop=True)
                nc.scalar.copy(zt[:, j*512:(j+1)*512], pz[:])
            if ow < 128:
                nc.vector.memset(yt[ow:128, :], 0.0)
            yts.append(yt)
            zts.append(zt)

        # distances + top-10 + wrapped idx
        idx_dram_q = [dram.tile([16, SQ], I16, name=f"idxq{q}") for q in range(NQ)]
        for q in range(NQ):
            idxT2 = wk.tile([16, TPQ * 128], I16, tag="idxT2", bufs=2)
            for tq in range(TPQ):
                t = q * TPQ + tq
                dt = wk.tile([128, N], F32, tag="dtile", bufs=2)
                for j in range(N // 512):
                    pd = ps.tile([128, 512], F32, tag="pd", bufs=2)
                    nc.tensor.matmul(pd[:], ones_r[:], negxxh[:, j*512:(j+1)*512],
                                     start=True, stop=False)
                    nc.tensor.matmul(pd[:], x[:, t*128:(t+1)*128],
                                     x[:, j*512:(j+1)*512], start=False, stop=True)
                    nc.scalar.copy(dt[:, j*512:(j+1)*512], pd[:])
                v8 = wk.tile([128, 8], F32, tag="v8", bufs=2)
                i8 = wk.tile([128, 8], U32, tag="i8", bufs=2)
                w8 = wk.tile([128, 8], F32, tag="w8", bufs=2)
                j8 = wk.tile([128, 8], U32, tag="j8", bufs=2)
                nc.vector.max(v8[:], dt[:])
                nc.vector.max_index(i8[:], v8[:], dt[:])
                nc.vector.match_replace(dt[:], v8[:], dt[:], NEG)
                nc.vector.max(w8[:], dt[:])
                nc.vector.max_index(j8[:], w8[:], dt[:])
                idxf = wk.tile([128, 16], F32, tag="idxf", bufs=2)
                nc.vector.tensor_copy(idxf[:, 0:8], i8[:])
                nc.vector.tensor_copy(idxf[:, 8:16], j8[:])
                pidx = ps.tile([16, 128], F32, tag="pidx", bufs=2)
                nc.tensor.transpose(pidx[:], idxf[:], ident[:])
                out_ap = bass.AP(idxT2.tensor, idxT2.offset + tq * 8,
                                 [[idxT2.ap[0][0], 10], [1, 8], [TPQ * 8, 16]])
                in_ap = bass.AP(pidx.tensor, pidx.offset,
                                [[pidx.ap[0][0], 10], [16, 8], [1, 16]])
                nc.scalar.copy(out_ap, in_ap)
            st_out = bass.AP(idx_dram_q[q].tensor, idx_dram_q[q].offset,
                             [[NPQ // 16, 10], [SQ, 16], [1, TPQ * 8]])
            nc.sync.dma_start(st_out, idxT2[0:10, :])

        idxw_tiles = []
        for q in range(NQ):
            idxw = wk.tile([128, SQ], I16, tag=f"idxw{q}")
            ld_in = bass.AP(idx_dram_q[q].tensor, idx_dram_q[q].offset,
                            [[0, 8], [SQ, 16], [1, SQ]])
            nc.sync.dma_start(idxw[:, :], ld_in)
            idxw_tiles.append(idxw)

        for oc in range(OC):
            ow = OCW[oc]
            yt, zt = yts[oc], zts[oc]
            mgz = wk.tile([ow, N], F32, tag="mgz", bufs=2)
            sg = wk.tile([ow, NQ], F32, tag="sg", bufs=2)
            sg2 = wk.tile([ow, NQ], F32, tag="sg2", bufs=2)
            cr = wk.tile([ow, NQ], F32, tag="cr", bufs=2)
            for q in range(NQ):
                g = wk.tile([128, KNN * NPQ], F32, tag="G", bufs=2)
                nc.gpsimd.ap_gather(g[:], yt[:], idxw_tiles[q][:],
                                    channels=128, num_elems=N, d=1,
                                    num_idxs=KNN * NPQ)
                gv = bass.AP(g.tensor, g.offset,
                             [[g.ap[0][0], ow], [1, NPQ], [NPQ, KNN]])
                nc.vector.tensor_reduce(mgz[:, q*NPQ:(q+1)*NPQ], gv,
                                        axis=mybir.AxisListType.X,
                                        op=mybir.AluOpType.max)
                s1 = wk.tile([ow, NPQ], F32, tag="s1", bufs=2)
                nc.vector.tensor_reduce(s1[:], gv, axis=mybir.AxisListType.X,
                                        op=mybir.AluOpType.add)
                nc.vector.tensor_reduce(sg[:, q:q+1], s1[:],
                                        axis=mybir.AxisListType.X,
                                        op=mybir.AluOpType.add)
                crt = wk.tile([ow, NPQ], F32, tag="crt", bufs=2)
                nc.vector.scalar_tensor_tensor(
                    crt[:], s1[:], 1.0, zt[:, q*NPQ:(q+1)*NPQ],
                    mybir.AluOpType.bypass, mybir.AluOpType.mult,
                    accum_out=cr[:, q:q+1])
                nc.scalar.activation(g[:ow, :], g[:ow, :],
                                     mybir.ActivationFunctionType.Square,
                                     accum_out=sg2[:, q:q+1])
            nc.vector.tensor_add(mgz[:], mgz[:], zt[:])
            nc.sync.dma_start(maxh_out[oc*128:oc*128+ow, :], mgz[:])
            sz = wk.tile([ow, 1], F32, tag="sz", bufs=2)
            nc.vector.tensor_reduce(sz[:], zt[:], axis=mybir.AxisListType.X,
                                    op=mybir.AluOpType.add)
            z2t = wk.tile([ow, N], F32, tag="z2t", bufs=2)
            sz2 = wk.tile([ow, 1], F32, tag="sz2", bufs=2)
            nc.vector.scalar_tensor_tensor(
                z2t[:], zt[:], 1.0, zt[:],
                mybir.AluOpType.bypass, mybir.AluOpType.mult, accum_out=sz2[:])
            ssg = wk.tile([ow, 1], F32, tag="ssg", bufs=2)
            nc.vector.tensor_reduce(ssg[:], sg[:], axis=mybir.AxisListType.X,
                                    op=mybir.AluOpType.add)
            ssg2 = wk.tile([ow, 1], F32, tag="ssg2", bufs=2)
            nc.vector.tensor_reduce(ssg2[:], sg2[:], axis=mybir.AxisListType.X,
                                    op=mybir.AluOpType.add)
            scr = wk.tile([ow, 1], F32, tag="scr", bufs=2)
            nc.vector.tensor_reduce(scr[:], cr[:], axis=mybir.AxisListType.X,
                                    op=mybir.AluOpType.add)
            p01 = wk.tile([ow, 2], F32, tag="p01", bufs=2)
            nc.vector.scalar_tensor_tensor(p01[:, 0:1], sz[:], float(KNN), ssg[:],
                                           mybir.AluOpType.mult,
                                           mybir.AluOpType.add)
            tmp1 = wk.tile([ow, 1], F32, tag="tmp1", bufs=2)
            nc.vector.scalar_tensor_tensor(tmp1[:], scr[:], 2.0, ssg2[:],
                                           mybir.AluOpType.mult,
                                           mybir.AluOpType.add)
            nc.vector.scalar_tensor_tensor(p01[:, 1:2], sz2[:], float(KNN), tmp1[:],
                                           mybir.AluOpType.mult,
                                           mybir.AluOpType.add)
            nc.sync.dma_start(part_out[oc*128:oc*128+ow, :], p01[:])
    nc.compile()
    return nc


def build_head():
    nc = bacc.Bacc("TRN2", target_bir_lowering=False, debug=False, num_devices=8)
    OS = [64, 64, 128, 256]
    ins = {}
    for i in range(4):
        ins[f"maxh{i}"] = nc.dram_tensor(f"maxh{i}", [OS[i], N], F32, kind="ExternalInput")
        ins[f"a{i}"] = nc.dram_tensor(f"a{i}", [OS[i], 1], F32, kind="ExternalInput")
        ins[f"b{i}"] = nc.dram_tensor(f"b{i}", [OS[i], 1], F32, kind="ExternalInput")
    # conv5 weight chunks aligned with cat structure (c-chunk rows x 256)
    CC5 = [64, 64, 128, 128, 128]
    w5_d = [nc.dram_tensor(f"w5c{i}", [CC5[i], 256], F32, kind="ExternalInput")
            for i in range(5)]
    w6_d = [nc.dram_tensor(f"w6c{i}", [128, 552], F32, kind="ExternalInput")
            for i in range(2)]
    CCD = [128, 128, 128, 128, 40]
    wd1_d = [nc.dram_tensor(f"wd1c{i}", [CCD[i], 552], F32, kind="ExternalInput")
             for i in range(5)]
    bd1_d = nc.dram_tensor("bd1", [552, 1], F32, kind="ExternalInput")
    CCD2 = [128, 128, 128, 128, 41]  # last chunk includes the bd2 bias row
    wd2_d = [nc.dram_tensor(f"wd2c{i}", [CCD2[i], 255], F32, kind="ExternalInput")
             for i in range(5)]

    feat_out = nc.dram_tensor("feature", [N, 552], F32, kind="ExternalOutput")
    log_out = nc.dram_tensor("logits", [N, 255], F32, kind="ExternalOutput")

    O6 = [128, 128, 128, 128, 40]    # conv6/dense1 o-chunks (552)

    with tile.TileContext(nc) as tc, ExitStack() as ctx:
        per = ctx.enter_context(tc.tile_pool(name="per", bufs=1))
        wk = ctx.enter_context(tc.tile_pool(name="wk", bufs=2))
        big = ctx.enter_context(tc.tile_pool(name="big", bufs=1))
        ps = ctx.enter_context(tc.tile_pool(name="ps", bufs=2, space="PSUM"))

        # x1..x4 reconstruction (x4 split into two 128-row tiles)
        xs = []   # list of (tile, rows) c-chunks in cat order
        for i in range(4):
            Oi = OS[i]
            nch = (Oi + 127) // 128
            for c in range(nch):
                cw = min(128, Oi - c * 128)
                av = per.tile([cw, 1], F32, name=f"av{i}_{c}")
                nc.sync.dma_start(av[:], ins[f"a{i}"][c*128:c*128+cw, :])
                bv = per.tile([cw, 1], F32, name=f"bv{i}_{c}")
                nc.sync.dma_start(bv[:], ins[f"b{i}"][c*128:c*128+cw, :])
                mh = wk.tile([cw, N], F32, tag="mht", bufs=2)
                nc.sync.dma_start(mh[:], ins[f"maxh{i}"][c*128:c*128+cw, :])
                xi = big.tile([128, N], F32, name=f"x{i}_{c}", tag="xs", bufs=13)[:cw, :]
                nc.vector.tensor_scalar(mh[:], mh[:], av[:], bv[:],
                                        mybir.AluOpType.mult, mybir.AluOpType.add)
                _leaky(nc, xi[:], mh[:])
                xs.append((xi, cw))

        w5 = [per.tile([CC5[i], 256], F32, name=f"w5_{i}") for i in range(5)]
        for i in range(5):
            nc.sync.dma_start(w5[i][:], w5_d[i][:, :])
        w6 = [per.tile([128, 552], F32, name=f"w6_{i}") for i in range(2)]
        for i in range(2):
            nc.sync.dma_start(w6[i][:], w6_d[i][:, :])
        wd1 = [per.tile([CCD[i], 552], F32, name=f"wd1_{i}") for i in range(5)]
        for i in range(5):
            nc.sync.dma_start(wd1[i][:], wd1_d[i][:, :])
        bd1 = per.tile([128, 5], F32)
        nc.sync.dma_start(bd1[:, 0:4], bd1_d[0:512, :].rearrange("(a c) o -> c (a o)", c=128))
        nc.sync.dma_start(bd1[0:40, 4:5], bd1_d[512:552, :])
        wd2 = [per.tile([CCD2[i], 255], F32, name=f"wd2_{i}") for i in range(5)]
        for i in range(5):
            nc.sync.dma_start(wd2[i][:], wd2_d[i][:, :])

        # conv5 -> h5 [2 x 128, N] with leaky
        h5 = [big.tile([128, N], F32, name=f"h5_{oc}", tag="xs", bufs=13) for oc in range(2)]
        for oc in range(2):
            for j in range(N // 512):
                p5 = ps.tile([128, 512], F32, tag="pp", bufs=3)
                for ci, (xi, cw) in enumerate(xs):
                    nc.tensor.matmul(p5[:], w5[ci][:, oc*128:(oc+1)*128],
                                     xi[:, j*512:(j+1)*512],
                                     start=(ci == 0), stop=(ci == len(xs) - 1))
                t5 = wk.tile([128, 512], F32, tag="t5", bufs=3)
                nc.scalar.copy(t5[:], p5[:])
                _leaky(nc, h5[oc][:, j*512:(j+1)*512], t5[:])

        # conv6 -> f feature-major (5 o-chunks) with leaky
        f = [big.tile([128, N], F32, name=f"f_{i}", tag="xs", bufs=13)[:O6[i] if i < 4 else 41, :] for i in range(5)]
        ooff = 0
        for oc in range(5):
            ow = O6[oc]
            for j in range(N // 512):
                p6 = ps.tile([ow, 512], F32, tag="pp", bufs=3)
                for ci in range(2):
                    nc.tensor.matmul(p6[:], w6[ci][:, ooff:ooff+ow],
                                     h5[ci][:, j*512:(j+1)*512],
                                     start=(ci == 0), stop=(ci == 1))
                t6 = wk.tile([ow, 512], F32, tag="t6", bufs=3)
                nc.scalar.copy(t6[:], p6[:])
                _leaky(nc, f[oc][:ow, j*512:(j+1)*512], t6[:])
            ooff += ow

        # feature point-major output: per n-tile, 2 column-halves of 552=276+276
        for t in range(NT):
            for hh in range(2):
                pf = ps.tile([128, 276], F32, tag="pp", bufs=3)
                for ci in range(2):
                    nc.tensor.matmul(pf[:], h5[ci][:, t*128:(t+1)*128],
                                     w6[ci][:, hh*276:(hh+1)*276],
                                     start=(ci == 0), stop=(ci == 1))
                tf = wk.tile([128, 276], F32, tag="tf", bufs=3)
                nc.scalar.copy(tf[:], pf[:])
                ff = wk.tile([128, 276], F32, tag="ff", bufs=3)
                _leaky(nc, ff[:], tf[:])
                nc.sync.dma_start(feat_out[t*128:(t+1)*128, hh*276:(hh+1)*276], ff[:])

        # dense1 -> d1 feature-major (5 o-chunks), bias bd1, no activation
        d1 = [big.tile([128, N], F32, name=f"d1_{i}", tag="xs", bufs=13)[:O6[i] if i < 4 else 41, :] for i in range(5)]
        ooff = 0
        for oc in range(5):
            ow = O6[oc]
            for j in range(N // 512):
                pD = ps.tile([ow, 512], F32, tag="pp", bufs=3)
                for ci in range(5):
                    cw = CCD[ci]
                    nc.tensor.matmul(pD[:], wd1[ci][:, ooff:ooff+ow],
                                     f[ci][:cw, j*512:(j+1)*512],
                                     start=(ci == 0), stop=(ci == 4))
                nc.scalar.activation(d1[oc][:ow, j*512:(j+1)*512], pD[:],
                                     mybir.ActivationFunctionType.Identity,
                                     bias=bd1[0:ow, oc:oc+1])
            ooff += ow

        # ones row into d1[4] row 40 (bias row pairing with wd2[4] row 40)
        ones_n = per.tile([1, N], F32)
        nc.vector.memset(ones_n[:], 1.0)
        nc.sync.dma_start(d1[4][40:41, :], ones_n[:])

        # dense2 point-major -> logits
        for t in range(NT):
            pL = ps.tile([128, 255], F32, tag="pp", bufs=3)
            for ci in range(5):
                cw = CCD2[ci]
                nc.tensor.matmul(pL[:], d1[ci][:cw, t*128:(t+1)*128],
                                 wd2[ci][:, :], start=(ci == 0), stop=(ci == 4))
            tL = wk.tile([128, 255], F32, tag="tL", bufs=3)
            nc.scalar.copy(tL[:], pL[:])
            nc.sync.dma_start(log_out[t*128:(t+1)*128, :], tL[:])
    nc.compile()
    return nc


# ---------------- host orchestration ----------------

_CACHE = {}


def _get_programs():
    if "stages" not in _CACHE:
        _CACHE["stages"] = [build_stage(L) for L in range(4)]
        _CACHE["head"] = build_head()
    return _CACHE["stages"], _CACHE["head"]


def _run(nc, in_maps):
    import os
    if os.environ.get("KERNEL_SIM"):
        from concourse.bass_interp import CoreSim
        out_names = []
        for alloc in nc.m.functions[0].allocations:
            if getattr(alloc, "kind", None) == "ExternalOutput":
                out_names.append(alloc.memorylocations[0].name)
        results = []
        for im in in_maps:
            sim = CoreSim(nc, trace=False, require_finite=False, require_nnan=False)
            for k, v in im.items():
                sim.tensor(k)[:] = v
            sim.simulate(check_with_hw=False)
            results.append({nm: np.array(sim.tensor(nm)) for nm in out_names})
        return results
    return _run_hw_cached(nc, in_maps)


_JIT_CACHE = {}


def _run_hw_cached(nc, in_maps):
    """Like bass2jax.run_bass_via_pjrt but caches the jitted executable."""
    import jax
    import numpy as np
    from jax.sharding import Mesh, PartitionSpec
    from jax.experimental.shard_map import shard_map
    from concourse import bass2jax, mybir as mb
    from concourse.bass2jax import _bass_exec_p, install_neuronx_cc_hook, partition_id_tensor

    key = id(nc)
    if key not in _JIT_CACHE:
        install_neuronx_cc_hook()
        pname = nc.partition_id_tensor.name if nc.partition_id_tensor else None
        in_names, out_names, out_avals, zero_shapes = [], [], [], []
        for alloc in nc.m.functions[0].allocations:
            if not isinstance(alloc, mb.MemoryLocationSet):
                continue
            name = alloc.memorylocations[0].name
            if alloc.kind == "ExternalInput":
                if name != pname:
                    in_names.append(name)
            elif alloc.kind == "ExternalOutput":
                out_names.append(name)
                shape = tuple(alloc.tensor_shape)
                dtype = mb.dt.np(alloc.dtype)
                out_avals.append(jax.core.ShapedArray(shape, dtype))
                zero_shapes.append((shape, dtype))
        n_params = len(in_names)
        all_names = in_names + out_names
        if pname is not None:
            all_names = all_names + [pname]

        def _body(*args):
            operands = list(args)
            if pname is not None:
                operands.append(partition_id_tensor())
            outs = _bass_exec_p.bind(
                *operands,
                out_avals=tuple(out_avals),
                in_names=tuple(all_names),
                out_names=tuple(out_names),
                lowering_input_output_aliases=(),
                sim_require_finite=True,
                sim_require_nnan=True,
                nc=nc,
            )
            return tuple(outs)

        devices = jax.devices()[:8]
        mesh = Mesh(np.asarray(devices), ("core",))
        n_outs = len(out_names)
        sharded = jax.jit(
            shard_map(_body, mesh=mesh,
                      in_specs=(PartitionSpec("core"),) * (n_params + n_outs),
                      out_specs=(PartitionSpec("core"),) * n_outs,
                      check_rep=False),
            donate_argnums=tuple(range(n_params, n_params + n_outs)),
            keep_unused=True,
        )
        _JIT_CACHE[key] = (sharded, in_names, out_names, out_avals, zero_shapes)

    sharded, in_names, out_names, out_avals, zero_shapes = _JIT_CACHE[key]
    concat_in = [np.concatenate([np.asarray(in_maps[c][nm]) for c in range(8)], axis=0)
                 for nm in in_names]
    concat_zeros = [np.zeros((8 * sh[0], *sh[1:]), dt) for sh, dt in zero_shapes]
    out_arrs = sharded(*concat_in, *concat_zeros)
    return [
        {nm: np.asarray(out_arrs[i]).reshape(8, *out_avals[i].shape)[c]
         for i, nm in enumerate(out_names)}
        for c in range(8)
    ]


def kernel(x, W1, g1, b1, W2, g2, b2, W3, g3, b3, W4, g4, b4, W5, W6,
           Wd1, bd1, Wd2, bd2):
    S, Bb, K, D = x.shape
    assert (S, Bb, K, D) == (2048, 8, 8, 3)
    pts = np.transpose(x, (1, 0, 2, 3)).reshape(Bb, S, K * D).astype(np.float32)
    ptsT = np.ascontiguousarray(np.transpose(pts, (0, 2, 1)))  # (B, 24, N)

    Ws = [W1, W2, W3, W4]
    gs = [g1, g2, g3, g4]
    bs = [b1, b2, b3, b4]
    ident = np.eye(128, dtype=np.float32)

    stages, head = _get_programs()

    maxhs = []   # per layer: (B, O, N)
    As, Bs2 = [], []
    mh_prev, a_prev, b_prev = None, None, None
    for L in range(4):
        C, O = LAYERS[L]
        W = Ws[L].astype(np.float32)
        Anb = np.ascontiguousarray(W[:, :C].T)            # (C, O)
        Bz = np.ascontiguousarray((W[:, C:] - W[:, :C]).T)
        in_maps = []
        for c in range(8):
            m = {"anb": Anb, "bz": Bz, "ident": ident}
            if L == 0:
                m["xin"] = np.ascontiguousarray(ptsT[c])
            else:
                m["maxh_prev"] = maxhs[L-1][c]
                m["a_prev"] = a_prev
                m["b_prev"] = b_prev
            in_maps.append(m)
        res = _run(stages[L], in_maps)
        mh = np.stack([res[c]["maxh"] for c in range(8)])  # (B, O, N)
        parts = np.stack([res[c]["part"] for c in range(8)]).astype(np.float64)
        M = float(B * N * KNN)
        s = parts[:, :, 0].sum(0)
        s2 = parts[:, :, 1].sum(0)
        mu = s / M
        var = s2 / M - mu * mu
        A = (gs[L].astype(np.float64) / np.sqrt(var + EPS))
        B2 = bs[L].astype(np.float64) - mu * A
        a_prev = A.astype(np.float32).reshape(O, 1)
        b_prev = B2.astype(np.float32).reshape(O, 1)
        As.append(a_prev)
        Bs2.append(b_prev)
        maxhs.append(mh)

    # head
    CC5 = [64, 64, 128, 128, 128]
    W5T = np.ascontiguousarray(W5.astype(np.float32).T)    # (512, 256)
    w5c = []
    off = 0
    for cw in CC5:
        w5c.append(np.ascontiguousarray(W5T[off:off+cw]))
        off += cw
    W6T = np.ascontiguousarray(W6.astype(np.float32).T)    # (256, 552)
    w6c = [np.ascontiguousarray(W6T[0:128]), np.ascontiguousarray(W6T[128:256])]
    Wd1T = np.ascontiguousarray(Wd1.astype(np.float32).T)  # (552, 552)
    CCD = [128, 128, 128, 128, 40]
    wd1c = []
    off = 0
    for cw in CCD:
        wd1c.append(np.ascontiguousarray(Wd1T[off:off+cw]))
        off += cw
    Wd2T = np.ascontiguousarray(Wd2.astype(np.float32).T)  # (552, 255)
    wd2c = []
    off = 0
    for cw in [128, 128, 128, 128, 40]:
        blk = Wd2T[off:off+cw]
        off += cw
        wd2c.append(np.ascontiguousarray(blk))
    wd2c[4] = np.ascontiguousarray(
        np.vstack([wd2c[4], bd2.astype(np.float32).reshape(1, 255)]))

    in_maps = []
    for c in range(8):
        m = {}
        for i in range(4):
            m[f"maxh{i}"] = maxhs[i][c]
            m[f"a{i}"] = As[i]
            m[f"b{i}"] = Bs2[i]
        for i in range(5):
            m[f"w5c{i}"] = w5c[i]
            m[f"wd1c{i}"] = wd1c[i]
            m[f"wd2c{i}"] = wd2c[i]
        for i in range(2):
            m[f"w6c{i}"] = w6c[i]
        m["bd1"] = bd1.astype(np.float32).reshape(552, 1)
        in_maps.append(m)
    res = _run(head, in_maps)
    feature = np.stack([res[c]["feature"] for c in range(8)])
    logits = np.stack([res[c]["logits"] for c in range(8)])
    return feature, logits


# revision 12
# speedup vs baseline: 1.5575x; 1.5575x over previous
"""DGCNN forward kernel for 8 Trainium2 NeuronCores (data-parallel over batch).

Five SPMD launches: one per EdgeConv layer (pairwise-dist matmul on PE,
exact top-10 via DVE max8/max_index/match_replace, neighbor gather via
gpsimd ap_gather, local BN partial sums) and one head launch
(cat -> conv5 -> conv6 -> dense1 -> dense2). Training-mode BN couples all
8 point clouds, so the tiny per-layer [O,2] partial sums are combined on
the host (float64) between launches.

EdgeConv algebra:
  h[n,kk,:] = Wnb x[idx[n,kk]] + (Wctr-Wnb) x[n]   (y-gather + z)
  max_k h = max_k(y[idx]) + z;  BN+LeakyReLU commute with max_k
  (BN scale gamma=1 > 0).
"""
import sys
import numpy as np

sys.path.insert(0, "/opt/trn_rl_repo")

from contextlib import ExitStack

import concourse.bass as bass
import concourse.bacc as bacc
import concourse.tile as tile
from concourse import mybir

F32 = mybir.dt.float32
I16 = mybir.dt.int16
U32 = mybir.dt.uint32

B = 8
N = 2048
NT = N // 128
KNN = 10
NQ = 4                   # gather quarters
TPQ = NT // NQ
NPQ = N // NQ
SQ = KNN * NPQ // 16     # wrapped idx cols per quarter
EPS = 1e-5
NEG = -1e30

LAYERS = [(24, 64), (64, 64), (64, 128), (128, 256)]


def _leaky(nc, out_ap, in_ap):
    nc.vector.scalar_tensor_tensor(out_ap, in_ap, 0.2, in_ap,
                                   mybir.AluOpType.mult, mybir.AluOpType.max)


def build_stage(L):
    C, O = LAYERS[L]
    OC = (O + 127) // 128
    OCW = [min(128, O - i * 128) for i in range(OC)]
    nc = bacc.Bacc("TRN2", target_bir_lowering=False, debug=False, num_devices=8)

    if L == 0:
        xin_d = nc.dram_tensor("xin", [C, N], F32, kind="ExternalInput")
    else:
        mh_d = nc.dram_tensor("maxh_prev", [C, N], F32, kind="ExternalInput")
        a_d = nc.dram_tensor("a_prev", [C, 1], F32, kind="ExternalInput")
        b_d = nc.dram_tensor("b_prev", [C, 1], F32, kind="ExternalInput")
    anb_d = nc.dram_tensor("anb", [C, O], F32, kind="ExternalInput")
    bz_d = nc.dram_tensor("bz", [C, O], F32, kind="ExternalInput")
    ident_d = nc.dram_tensor("ident", [128, 128], F32, kind="ExternalInput")

    maxh_out = nc.dram_tensor("maxh", [O, N], F32, kind="ExternalOutput")
    part_out = nc.dram_tensor("part", [O, 2], F32, kind="ExternalOutput")

    with tile.TileContext(nc) as tc, ExitStack() as ctx:
        per = ctx.enter_context(tc.tile_pool(name="per", bufs=1))
        wk = ctx.enter_context(tc.tile_pool(name="wk", bufs=1))
        dram = ctx.enter_context(tc.tile_pool(name="dram", bufs=1, space="DRAM"))
        ps = ctx.enter_context(tc.tile_pool(name="ps", bufs=2, space="PSUM"))

        x = per.tile([C, N], F32)
        if L == 0:
            nc.sync.dma_start(x[:], xin_d[:, :])
        else:
            mh = per.tile([C, N], F32)
            nc.sync.dma_start(mh[:], mh_d[:, :])
            av = per.tile([C, 1], F32)
            nc.sync.dma_start(av[:], a_d[:, :])
            bv = per.tile([C, 1], F32)
            nc.sync.dma_start(bv[:], b_d[:, :])
            xt = per.tile([C, N], F32)
            nc.vector.tensor_scalar(xt[:], mh[:], av[:], bv[:],
                                    mybir.AluOpType.mult, mybir.AluOpType.add)
            _leaky(nc, x[:], xt[:])

        anb = per.tile([C, O], F32)
        nc.sync.dma_start(anb[:], anb_d[:, :])
        bz = per.tile([C, O], F32)
        nc.sync.dma_start(bz[:], bz_d[:, :])
        ident = per.tile([128, 128], F32)
        nc.sync.dma_start(ident[:], ident_d[:, :])

        x2 = per.tile([C, N], F32)
        nc.vector.tensor_mul(x2[:], x[:], x[:])
        ones_c = per.tile([C, 1], F32)
        nc.vector.memset(ones_c[:], 1.0)
        ones_r = per.tile([1, 128], F32)
        nc.vector.memset(ones_r[:], 1.0)
        negxxh = per.tile([1, N], F32)
        for j in range(N // 512):
            pxx = ps.tile([1, 512], F32, tag="py", bufs=2)
            nc.tensor.matmul(pxx[:], ones_c[:],
                             x2[:, j*512:(j+1)*512], start=True, stop=True)
            nc.scalar.activation(negxxh[:, j*512:(j+1)*512], pxx[:],
                                 mybir.ActivationFunctionType.Copy, scale=-0.5)

        # y / z feature-major [O, N]
        yts, zts = [], []
        for oc in range(OC):
            ow = OCW[oc]
            yt = per.tile([128, N], F32, name=f"yt{oc}")
            zt = per.tile([ow, N], F32, name=f"zt{oc}")
            for j in range(N // 512):
                py = ps.tile([ow, 512], F32, tag="py", bufs=2)
                nc.tensor.matmul(py[:], anb[:, oc*128:oc*128+ow],
                                 x[:, j*512:(j+1)*512], start=True, stop=True)
                nc.scalar.copy(yt[:ow, j*512:(j+1)*512], py[:])
                pz = ps.tile([ow, 512], F32, tag="py", bufs=2)
                nc.tensor.matmul(pz[:], bz[:, oc*128:oc*128+ow],
                                 x[:, j*512:(j+1)*512], start=True, stop=True)
                nc.scalar.copy(zt[:, j*512:(j+1)*512], pz[:])
            if ow < 128:
                nc.vector.memset(yt[ow:128, :], 0.0)
            yts.append(yt)
            zts.append(zt)

        # distances + top-10 + wrapped idx
        idx_dram_q = [dram.tile([16, SQ], I16, name=f"idxq{q}") for q in range(NQ)]
        for q in range(NQ):
            idxT2 = wk.tile([16, TPQ * 128], I16, tag="idxT2", bufs=2)
            for tq in range(TPQ):
                t = q * TPQ + tq
                dt = wk.tile([128, N], F32, tag="dtile", bufs=2)
                for j in range(N // 512):
                    pd = ps.tile([128, 512], F32, tag="pd", bufs=2)
                    nc.tensor.matmul(pd[:], ones_r[:], negxxh[:, j*512:(j+1)*512],
                                     start=True, stop=False)
                    nc.tensor.matmul(pd[:], x[:, t*128:(t+1)*128],
                                     x[:, j*512:(j+1)*512], start=False, stop=True)
                    nc.scalar.copy(dt[:, j*512:(j+1)*512], pd[:])
                v8 = wk.tile([128, 8], F32, tag="v8", bufs=2)
                i8 = wk.tile([128, 8], U32, tag="i8", bufs=2)
                w8 = wk.tile([128, 8], F32, tag="w8", bufs=2)
                j8 = wk.tile([128, 8], U32, tag="j8", bufs=2)
                nc.vector.max(v8[:], dt[:])
                nc.vector.max_index(i8[:], v8[:], dt[:])
                nc.vector.match_replace(dt[:], v8[:], dt[:], NEG)
                nc.vector.max(w8[:], dt[:])
                nc.vector.max_index(j8[:], w8[:], dt[:])
                idxf = wk.tile([128, 16], F32, tag="idxf", bufs=2)
                nc.vector.tensor_copy(idxf[:, 0:8], i8[:])
                nc.vector.tensor_copy(idxf[:, 8:16], j8[:])
                pidx = ps.tile([16, 128], F32, tag="pidx", bufs=2)
                nc.tensor.transpose(pidx[:], idxf[:], ident[:])
                out_ap = bass.AP(idxT2.tensor, idxT2.offset + tq * 8,
                                 [[idxT2.ap[0][0], 10], [1, 8], [TPQ * 8, 16]])
                in_ap = bass.AP(pidx.tensor, pidx.offset,
                                [[pidx.ap[0][0], 10], [16, 8], [1, 16]])
                nc.scalar.copy(out_ap, in_ap)
            st_out = bass.AP(idx_dram_q[q].tensor, idx_dram_q[q].offset,
                             [[NPQ // 16, 10], [SQ, 16], [1, TPQ * 8]])
            nc.sync.dma_start(st_out, idxT2[0:10, :])

        idxw_tiles = []
        for q in range(NQ):
            idxw = wk.tile([128, SQ], I16, tag=f"idxw{q}")
            ld_in = bass.AP(idx_dram_q[q].tensor, idx_dram_q[q].offset,
                            [[0, 8], [SQ, 16], [1, SQ]])
            nc.sync.dma_start(idxw[:, :], ld_in)
            idxw_tiles.append(idxw)

        for oc in range(OC):
            ow = OCW[oc]
            yt, zt = yts[oc], zts[oc]
            mgz = wk.tile([ow, N], F32, tag="mgz", bufs=2)
            sg = wk.tile([ow, NQ], F32, tag="sg", bufs=2)
            sg2 = wk.tile([ow, NQ], F32, tag="sg2", bufs=2)
            cr = wk.tile([ow, NQ], F32, tag="cr", bufs=2)
            for q in range(NQ):
                g = wk.tile([128, KNN * NPQ], F32, tag="G", bufs=2)
                nc.gpsimd.ap_gather(g[:], yt[:], idxw_tiles[q][:],
                                    channels=128, num_elems=N, d=1,
                                    num_idxs=KNN * NPQ)
                gv = bass.AP(g.tensor, g.offset,
                             [[g.ap[0][0], ow], [1, NPQ], [NPQ, KNN]])
                nc.vector.tensor_reduce(mgz[:, q*NPQ:(q+1)*NPQ], gv,
                                        axis=mybir.AxisListType.X,
                                        op=mybir.AluOpType.max)
                s1 = wk.tile([ow, NPQ], F32, tag="s1", bufs=2)
                nc.vector.tensor_reduce(s1[:], gv, axis=mybir.AxisListType.X,
                                        op=mybir.AluOpType.add)
                nc.vector.tensor_reduce(sg[:, q:q+1], s1[:],
                                        axis=mybir.AxisListType.X,
                                        op=mybir.AluOpType.add)
                crt = wk.tile([ow, NPQ], F32, tag="crt", bufs=2)
                nc.vector.scalar_tensor_tensor(
                    crt[:], s1[:], 1.0, zt[:, q*NPQ:(q+1)*NPQ],
                    mybir.AluOpType.bypass, mybir.AluOpType.mult,
                    accum_out=cr[:, q:q+1])
                nc.scalar.activation(g[:ow, :], g[:ow, :],
                                     mybir.ActivationFunctionType.Square,
                                     accum_out=sg2[:, q:q+1])
            nc.vector.tensor_add(mgz[:], mgz[:], zt[:])
            nc.sync.dma_start(maxh_out[oc*128:oc*128+ow, :], mgz[:])
            sz = wk.tile([ow, 1], F32, tag="sz", bufs=2)
            nc.vector.tensor_reduce(sz[:], zt[:], axis=mybir.AxisListType.X,
                                    op=mybir.AluOpType.add)
            z2t = wk.tile([ow, N], F32, tag="z2t", bufs=2)
            sz2 = wk.tile([ow, 1], F32, tag="sz2", bufs=2)
            nc.vector.scalar_tensor_tensor(
                z2t[:], zt[:], 1.0, zt[:],
                mybir.AluOpType.bypass, mybir.AluOpType.mult, accum_out=sz2[:])
            ssg = wk.tile([ow, 1], F32, tag="ssg", bufs=2)
            nc.vector.tensor_reduce(ssg[:], sg[:], axis=mybir.AxisListType.X,
                                    op=mybir.AluOpType.add)
            ssg2 = wk.tile([ow, 1], F32, tag="ssg2", bufs=2)
            nc.vector.tensor_reduce(ssg2[:], sg2[:], axis=mybir.AxisListType.X,
                                    op=mybir.AluOpType.add)
            scr = wk.tile([ow, 1], F32, tag="scr", bufs=2)
            nc.vector.tensor_reduce(scr[:], cr[:], axis=mybir.AxisListType.X,
                                    op=mybir.AluOpType.add)
            p01 = wk.tile([ow, 2], F32, tag="p01", bufs=2)
            nc.vector.scalar_tensor_tensor(p01[:, 0:1], sz[:], float(KNN), ssg[:],
                                           mybir.AluOpType.mult,
                                           mybir.AluOpType.add)
            tmp1 = wk.tile([ow, 1], F32, tag="tmp1", bufs=2)
            nc.vector.scalar_tensor_tensor(tmp1[:], scr[:], 2.0, ssg2[:],
                                           mybir.AluOpType.mult,
                                           mybir.AluOpType.add)
            nc.vector.scalar_tensor_tensor(p01[:, 1:2], sz2[:], float(KNN), tmp1[:],
                                           mybir.AluOpType.mult,
                                           mybir.AluOpType.add)
            nc.sync.dma_start(part_out[oc*128:oc*128+ow, :], p01[:])
    nc.compile()
    return nc


def build_head():
    nc = bacc.Bacc("TRN2", target_bir_lowering=False, debug=False, num_devices=8)
    OS = [64, 64, 128, 256]
    ins = {}
    for i in range(4):
        ins[f"maxh{i}"] = nc.dram_tensor(f"maxh{i}", [OS[i], N], F32, kind="ExternalInput")
        ins[f"a{i}"] = nc.dram_tensor(f"a{i}", [OS[i], 1], F32, kind="ExternalInput")
        ins[f"b{i}"] = nc.dram_tensor(f"b{i}", [OS[i], 1], F32, kind="ExternalInput")
    # conv5 weight chunks aligned with cat structure (c-chunk rows x 256)
    CC5 = [64, 64, 128, 128, 128]
    w5_d = [nc.dram_tensor(f"w5c{i}", [CC5[i], 256], F32, kind="ExternalInput")
            for i in range(5)]
    w6_d = [nc.dram_tensor(f"w6c{i}", [128, 552], F32, kind="ExternalInput")
            for i in range(2)]
    CCD = [128, 128, 128, 128, 40]
    wd1_d = [nc.dram_tensor(f"wd1c{i}", [CCD[i], 552], F32, kind="ExternalInput")
             for i in range(5)]
    bd1_d = nc.dram_tensor("bd1", [552, 1], F32, kind="ExternalInput")
    CCD2 = [128, 128, 128, 128, 41]  # last chunk includes the bd2 bias row
    wd2_d = [nc.dram_tensor(f"wd2c{i}", [CCD2[i], 255], F32, kind="ExternalInput")
             for i in range(5)]

    feat_out = nc.dram_tensor("feature", [N, 552], F32, kind="ExternalOutput")
    log_out = nc.dram_tensor("logits", [N, 255], F32, kind="ExternalOutput")

    O6 = [128, 128, 128, 128, 40]    # conv6/dense1 o-chunks (552)

    with tile.TileContext(nc) as tc, ExitStack() as ctx:
        per = ctx.enter_context(tc.tile_pool(name="per", bufs=1))
        wk = ctx.enter_context(tc.tile_pool(name="wk", bufs=2))
        big = ctx.enter_context(tc.tile_pool(name="big", bufs=1))
        ps = ctx.enter_context(tc.tile_pool(name="ps", bufs=2, space="PSUM"))

        # x1..x4 reconstruction (x4 split into two 128-row tiles)
        xs = []   # list of (tile, rows) c-chunks in cat order
        for i in range(4):
            Oi = OS[i]
            nch = (Oi + 127) // 128
            for c in range(nch):
                cw = min(128, Oi - c * 128)
                av = per.tile([cw, 1], F32, name=f"av{i}_{c}")
                nc.sync.dma_start(av[:], ins[f"a{i}"][c*128:c*128+cw, :])
                bv = per.tile([cw, 1], F32, name=f"bv{i}_{c}")
                nc.sync.dma_start(bv[:], ins[f"b{i}"][c*128:c*128+cw, :])
                mh = wk.tile([cw, N], F32, tag="mht", bufs=2)
                nc.sync.dma_start(mh[:], ins[f"maxh{i}"][c*128:c*128+cw, :])
                xi = big.tile([128, N], F32, name=f"x{i}_{c}", tag="xs", bufs=13)[:cw, :]
                nc.vector.tensor_scalar(mh[:], mh[:], av[:], bv[:],
                                        mybir.AluOpType.mult, mybir.AluOpType.add)
                _leaky(nc, xi[:], mh[:])
                xs.append((xi, cw))

        w5 = [per.tile([CC5[i], 256], F32, name=f"w5_{i}") for i in range(5)]
        for i in range(5):
            nc.sync.dma_start(w5[i][:], w5_d[i][:, :])
        w6 = [per.tile([128, 552], F32, name=f"w6_{i}") for i in range(2)]
        for i in range(2):
            nc.sync.dma_start(w6[i][:], w6_d[i][:, :])
        wd1 = [per.tile([CCD[i], 552], F32, name=f"wd1_{i}") for i in range(5)]
        for i in range(5):
            nc.sync.dma_start(wd1[i][:], wd1_d[i][:, :])
        bd1 = per.tile([128, 5], F32)
        nc.sync.dma_start(bd1[:, 0:4], bd1_d[0:512, :].rearrange("(a c) o -> c (a o)", c=128))
        nc.sync.dma_start(bd1[0:40, 4:5], bd1_d[512:552, :])
        wd2 = [per.tile([CCD2[i], 255], F32, name=f"wd2_{i}") for i in range(5)]
        for i in range(5):
            nc.sync.dma_start(wd2[i][:], wd2_d[i][:, :])

        # conv5 -> h5 [2 x 128, N] with leaky
        h5 = [big.tile([128, N], F32, name=f"h5_{oc}", tag="xs", bufs=13) for oc in range(2)]
        for oc in range(2):
            for j in range(N // 512):
                p5 = ps.tile([128, 512], F32, tag="pp", bufs=3)
                for ci, (xi, cw) in enumerate(xs):
                    nc.tensor.matmul(p5[:], w5[ci][:, oc*128:(oc+1)*128],
                                     xi[:, j*512:(j+1)*512],
                                     start=(ci == 0), stop=(ci == len(xs) - 1))
                t5 = wk.tile([128, 512], F32, tag="t5", bufs=3)
                nc.scalar.copy(t5[:], p5[:])
                _leaky(nc, h5[oc][:, j*512:(j+1)*512], t5[:])

        # conv6 -> f feature-major (5 o-chunks) with leaky
        f = [big.tile([128, N], F32, name=f"f_{i}", tag="xs", bufs=13)[:O6[i] if i < 4 else 41, :] for i in range(5)]
        ooff = 0
        for oc in range(5):
            ow = O6[oc]
            for j in range(N // 512):
                p6 = ps.tile([ow, 512], F32, tag="pp", bufs=3)
                for ci in range(2):
                    nc.tensor.matmul(p6[:], w6[ci][:, ooff:ooff+ow],
                                     h5[ci][:, j*512:(j+1)*512],
                                     start=(ci == 0), stop=(ci == 1))
                t6 = wk.tile([ow, 512], F32, tag="t6", bufs=3)
                nc.scalar.copy(t6[:], p6[:])
                _leaky(nc, f[oc][:ow, j*512:(j+1)*512], t6[:])
            ooff += ow

        # feature point-major output: per n-tile, 2 column-halves of 552=276+276
        for t in range(NT):
            for hh in range(2):
                pf = ps.tile([128, 276], F32, tag="pp", bufs=3)
                for ci in range(2):
                    nc.tensor.matmul(pf[:], h5[ci][:, t*128:(t+1)*128],
                                     w6[ci][:, hh*276:(hh+1)*276],
                                     start=(ci == 0), stop=(ci == 1))
                tf = wk.tile([128, 276], F32, tag="tf", bufs=3)
                nc.scalar.copy(tf[:], pf[:])
                ff = wk.tile([128, 276], F32, tag="ff", bufs=3)
                _leaky(nc, ff[:], tf[:])
                nc.sync.dma_start(feat_out[t*128:(t+1)*128, hh*276:(hh+1)*276], ff[:])

        # dense1 -> d1 feature-major (5 o-chunks), bias bd1, no activation
        d1 = [big.tile([128, N], F32, name=f"d1_{i}", tag="xs", bufs=13)[:O6[i] if i < 4 else 41, :] for i in range(5)]
        ooff = 0
        for oc in range(5):
            ow = O6[oc]
            for j in range(N // 512):
                pD = ps.tile([ow, 512], F32, tag="pp", bufs=3)
                for ci in range(5):
                    cw = CCD[ci]
                    nc.tensor.matmul(pD[:], wd1[ci][:, ooff:ooff+ow],
                                     f[ci][:cw, j*512:(j+1)*512],
                                     start=(ci == 0), stop=(ci == 4))
                nc.scalar.activation(d1[oc][:ow, j*512:(j+1)*512], pD[:],
                                     mybir.ActivationFunctionType.Identity,
                                     bias=bd1[0:ow, oc:oc+1])
            ooff += ow

        # ones row into d1[4] row 40 (bias row pairing with wd2[4] row 40)
        ones_n = per.tile([1, N], F32)
        nc.vector.memset(ones_n[:], 1.0)
        nc.sync.dma_start(d1[4][40:41, :], ones_n[:])

        # dense2 point-major -> logits
        for t in range(NT):
            pL = ps.tile([128, 255], F32, tag="pp", bufs=3)
            for ci in range(5):
                cw = CCD2[ci]
                nc.tensor.matmul(pL[:], d1[ci][:cw, t*128:(t+1)*128],
                                 wd2[ci][:, :], start=(ci == 0), stop=(ci == 4))
            tL = wk.tile([128, 255], F32, tag="tL", bufs=3)
            nc.scalar.copy(tL[:], pL[:])
            nc.sync.dma_start(log_out[t*128:(t+1)*128, :], tL[:])
    nc.compile()
    return nc


# ---------------- host orchestration ----------------

_CACHE = {}


def _get_programs():
    if "stages" not in _CACHE:
        _CACHE["stages"] = [build_stage(L) for L in range(4)]
        _CACHE["head"] = build_head()
    return _CACHE["stages"], _CACHE["head"]


def _run(nc, in_maps):
    import os
    if os.environ.get("KERNEL_SIM"):
        from concourse.bass_interp import CoreSim
        out_names = []
        for alloc in nc.m.functions[0].allocations:
            if getattr(alloc, "kind", None) == "ExternalOutput":
                out_names.append(alloc.memorylocations[0].name)
        results = []
        for im in in_maps:
            sim = CoreSim(nc, trace=False, require_finite=False, require_nnan=False)
            for k, v in im.items():
                v = np.asarray(v)
                dst = sim.tensor(k)
                if v.shape != dst.shape:  # global array: take this core's shard
                    per = v.shape[0] // 8
                    v = v[len(results)*per:(len(results)+1)*per]
                dst[:] = v
            sim.simulate(check_with_hw=False)
            results.append({nm: np.array(sim.tensor(nm)) for nm in out_names})
        raw = {nm: np.concatenate([r[nm] for r in results], axis=0) for nm in out_names}
        return results, raw
    return _run_hw_cached(nc, in_maps)


_JIT_CACHE = {}


class _LazyResults:
    """Per-core view of global outputs; materializes to host on access."""
    def __init__(self, out_arrs, out_names, out_avals):
        self._arrs = dict(zip(out_names, out_arrs))
        self._avals = dict(zip(out_names, out_avals))
        self._np = {}

    def __getitem__(self, c):
        return _LazyCore(self, c)


class _LazyCore:
    def __init__(self, parent, c):
        self._p = parent
        self._c = c

    def __getitem__(self, nm):
        if nm not in self._p._np:
            a = np.asarray(self._p._arrs[nm])
            self._p._np[nm] = a.reshape(8, *self._p._avals[nm].shape)
        return self._p._np[nm][self._c]


def _run_hw_cached(nc, in_maps):
    """Like bass2jax.run_bass_via_pjrt but caches the jitted executable."""
    import jax
    import numpy as np
    from jax.sharding import Mesh, PartitionSpec
    from jax.experimental.shard_map import shard_map
    from concourse import bass2jax, mybir as mb
    from concourse.bass2jax import _bass_exec_p, install_neuronx_cc_hook, partition_id_tensor

    key = id(nc)
    if key not in _JIT_CACHE:
        install_neuronx_cc_hook()
        pname = nc.partition_id_tensor.name if nc.partition_id_tensor else None
        in_names, out_names, out_avals, zero_shapes = [], [], [], []
        for alloc in nc.m.functions[0].allocations:
            if not isinstance(alloc, mb.MemoryLocationSet):
                continue
            name = alloc.memorylocations[0].name
            if alloc.kind == "ExternalInput":
                if name != pname:
                    in_names.append(name)
            elif alloc.kind == "ExternalOutput":
                out_names.append(name)
                shape = tuple(alloc.tensor_shape)
                dtype = mb.dt.np(alloc.dtype)
                out_avals.append(jax.core.ShapedArray(shape, dtype))
                zero_shapes.append((shape, dtype))
        n_params = len(in_names)
        all_names = in_names + out_names
        if pname is not None:
            all_names = all_names + [pname]

        def _body(*args):
            operands = list(args)
            if pname is not None:
                operands.append(partition_id_tensor())
            outs = _bass_exec_p.bind(
                *operands,
                out_avals=tuple(out_avals),
                in_names=tuple(all_names),
                out_names=tuple(out_names),
                lowering_input_output_aliases=(),
                sim_require_finite=True,
                sim_require_nnan=True,
                nc=nc,
            )
            return tuple(outs)

        devices = jax.devices()[:8]
        mesh = Mesh(np.asarray(devices), ("core",))
        n_outs = len(out_names)
        sharded = jax.jit(
            shard_map(_body, mesh=mesh,
                      in_specs=(PartitionSpec("core"),) * (n_params + n_outs),
                      out_specs=(PartitionSpec("core"),) * n_outs,
                      check_rep=False),
            donate_argnums=tuple(range(n_params, n_params + n_outs)),
            keep_unused=True,
        )
        _JIT_CACHE[key] = (sharded, in_names, out_names, out_avals, zero_shapes)

    sharded, in_names, out_names, out_avals, zero_shapes = _JIT_CACHE[key]
    import jax
    concat_in = []
    for nm in in_names:
        v0 = in_maps[0][nm]
        if isinstance(v0, jax.Array):
            concat_in.append(v0)  # already global/sharded on device
        else:
            concat_in.append(np.concatenate(
                [np.asarray(in_maps[c][nm]) for c in range(8)], axis=0))
    concat_zeros = [np.zeros((8 * sh[0], *sh[1:]), dt) for sh, dt in zero_shapes]
    out_arrs = sharded(*concat_in, *concat_zeros)
    raw = {nm: out_arrs[i] for i, nm in enumerate(out_names)}
    percore = [
        {nm: out_arrs[i] for i, nm in enumerate(out_names)}
        for c in range(8)
    ]
    return _LazyResults(out_arrs, out_names, out_avals), raw


def kernel(x, W1, g1, b1, W2, g2, b2, W3, g3, b3, W4, g4, b4, W5, W6,
           Wd1, bd1, Wd2, bd2):
    S, Bb, K, D = x.shape
    assert (S, Bb, K, D) == (2048, 8, 8, 3)
    pts = np.transpose(x, (1, 0, 2, 3)).reshape(Bb, S, K * D).astype(np.float32)
    ptsT = np.ascontiguousarray(np.transpose(pts, (0, 2, 1)))  # (B, 24, N)

    Ws = [W1, W2, W3, W4]
    gs = [g1, g2, g3, g4]
    bs = [b1, b2, b3, b4]
    ident = np.eye(128, dtype=np.float32)

    stages, head = _get_programs()

    maxhs = []   # per layer: (B, O, N)
    As, Bs2 = [], []
    mh_prev, a_prev, b_prev = None, None, None
    for L in range(4):
        C, O = LAYERS[L]
        W = Ws[L].astype(np.float32)
        Anb = np.ascontiguousarray(W[:, :C].T)            # (C, O)
        Bz = np.ascontiguousarray((W[:, C:] - W[:, :C]).T)
        in_maps = []
        for c in range(8):
            m = {"anb": Anb, "bz": Bz, "ident": ident}
            if L == 0:
                m["xin"] = np.ascontiguousarray(ptsT[c])
            else:
                m["maxh_prev"] = maxhs[L-1]
                m["a_prev"] = a_prev
                m["b_prev"] = b_prev
            in_maps.append(m)
        res, raw = _run(stages[L], in_maps)
        mh = raw["maxh"]  # global (8*O, N), device-resident on HW path
        parts = np.stack([res[c]["part"] for c in range(8)]).astype(np.float64)
        M = float(B * N * KNN)
        s = parts[:, :, 0].sum(0)
        s2 = parts[:, :, 1].sum(0)
        mu = s / M
        var = s2 / M - mu * mu
        A = (gs[L].astype(np.float64) / np.sqrt(var + EPS))
        B2 = bs[L].astype(np.float64) - mu * A
        a_prev = A.astype(np.float32).reshape(O, 1)
        b_prev = B2.astype(np.float32).reshape(O, 1)
        As.append(a_prev)
        Bs2.append(b_prev)
        maxhs.append(mh)

    # head
    CC5 = [64, 64, 128, 128, 128]
    W5T = np.ascontiguousarray(W5.astype(np.float32).T)    # (512, 256)
    w5c = []
    off = 0
    for cw in CC5:
        w5c.append(np.ascontiguousarray(W5T[off:off+cw]))
        off += cw
    W6T = np.ascontiguousarray(W6.astype(np.float32).T)    # (256, 552)
    w6c = [np.ascontiguousarray(W6T[0:128]), np.ascontiguousarray(W6T[128:256])]
    Wd1T = np.ascontiguousarray(Wd1.astype(np.float32).T)  # (552, 552)
    CCD = [128, 128, 128, 128, 40]
    wd1c = []
    off = 0
    for cw in CCD:
        wd1c.append(np.ascontiguousarray(Wd1T[off:off+cw]))
        off += cw
    Wd2T = np.ascontiguousarray(Wd2.astype(np.float32).T)  # (552, 255)
    wd2c = []
    off = 0
    for cw in [128, 128, 128, 128, 40]:
        blk = Wd2T[off:off+cw]
        off += cw
        wd2c.append(np.ascontiguousarray(blk))
    wd2c[4] = np.ascontiguousarray(
        np.vstack([wd2c[4], bd2.astype(np.float32).reshape(1, 255)]))

    in_maps = []
    for c in range(8):
        m = {}
        for i in range(4):
            m[f"maxh{i}"] = maxhs[i]
            m[f"a{i}"] = As[i]
            m[f"b{i}"] = Bs2[i]
        for i in range(5):
            m[f"w5c{i}"] = w5c[i]
            m[f"wd1c{i}"] = wd1c[i]
            m[f"wd2c{i}"] = wd2c[i]
        for i in range(2):
            m[f"w6c{i}"] = w6c[i]
        m["bd1"] = bd1.astype(np.float32).reshape(552, 1)
        in_maps.append(m)
    res, raw = _run(head, in_maps)
    feature = np.asarray(raw["feature"]).reshape(8, N, 552)
    logits = np.asarray(raw["logits"]).reshape(8, N, 255)
    return feature, logits
